# revision 34
# baseline (speedup 1.0000x reference)
"""LoFTR cross-attention on 8 Trainium2 NeuronCores.

Problem: x [2, 4096, 256], source [2, 6144, 256], Wq/Wk/Wv [256, 256] (torch
Linear convention, y = x @ W.T), 8 heads x 32 dims, softmax cross-attention,
output [2, 4096, 256] fp32.

Sharding: 16 (batch, head) pairs over 8 cores -> each core owns one batch b
and two adjacent heads {2p, 2p+1}. Per core:
  q = x[b] @ Wq_h.T        [4096, 32] per head
  k = source[b] @ Wk_h.T   [6144, 32]
  v = source[b] @ Wv_h.T   [6144, 32]
  out = softmax(q k^T / sqrt(32)) v

Kernel layout strategy (all matmuls bf16 into fp32 PSUM):
  - Host passes x^T and source^T (plus head-sliced, replicated weight packs) so
    every on-chip matmul has its contraction dim on partitions; no on-chip
    transposes of the big tensors.
  - Scores are computed TRANSPOSED: s^T[kpos, qpos] = k @ q^T via PE with
    K=32-row-tiling: 3 concurrent matmuls in partition strips {0,32,64}
    (tile_position), writing 3 PSUM banks.
  - The softmax exp is the wall: 50.3M exps/core. Each score group lands in
    TWO separate PSUM tiles (2-bank + 1-bank); ScalarE (exact ACT exp) and
    VectorE drain one tile each, concurrently (separate tiles because the
    Tile dep engine serializes same-tile readers). The DVE side is a single
    tensor_scalar computing a Schraudolph fast-exp: i16 = int16(s * SCALE *
    128*log2(e) + B) whose bits ARE the bf16 exp value (written through
    AP.bitcast). Its ~1.7% rms error (constants fitted to the score
    distribution) washes out in the softmax average over 6144 keys; the 2,1
    alternating k-chunk split gives DVE a uniform ~48% of keys; measured
    end-to-end rel err 1.3e-2. The previous chunk's attnv matmuls are
    interleaved between score groups so the in-order PE fills exp-wait gaps,
    and the next chunk's q projection is hoisted into the middle of each
    chunk to kill the boundary bubble.
  - attn @ v is 2-way COLUMN-tiled on the PE: strips at tile_position (0,0)
    and (0,64), each with stationary [v_m | 1] (33 cols; the ones column folds
    the softmax denominator into PSUM rows 32/96). Even k-chunks go to strip
    0, odd to strip 1; the strips run concurrently, halving attnv PE time.
  - The strip partials (PSUM partitions 0:33 / 64:97) are copied to SBUF bf16
    (one on ScalarE, one on VectorE), then summed + transposed in one step by
    accumulating two identity matmuls per 128-query block into the same PSUM
    (strip 1 reads at base partition 64 -> PE row-tile (64, 0)).
  - One strided DVE reciprocal + 4 tensor_scalar_mul normalize and land the
    output in natural [qpos, dhead] layout for one big DMA out.
"""

import numpy as np

B = 2
L1 = 4096
L2 = 6144
D = 256
NHEAD = 8
DH = 32
HEADS_PER_CORE = 2
N_CORES = 8
QB = 512                 # query block (free dim of scores matmuls)
NQ = L1 // QB            # 8 query chunks
NK = L2 // 128           # 48 key chunks of 128
RT = 3                   # row-tiling ways for K=32 scores matmuls
NG = NK // RT            # 16 groups of 3 key chunks
SCALE = 1.0 / np.sqrt(DH)
VW = DH + 1              # v columns + ones column

# Schraudolph fast-exp constants (fit to the actual score distribution):
# i16 = int16(s_raw * (SCALE * 128*log2 e) + SB_C); bits viewed as bf16.
SA = 184.6649652337873   # 128 * log2(e)
SB_C = 16250.5           # 127*128 with fitted offset (robust to trunc/round)

# exp engine split, by whole k-chunks of each [128, 3*512] score group:
# ScalarE (exact exp) takes the first `split` k-chunks, VectorE (Schraudolph)
# the rest. Each group's scores land in TWO separate PSUM tiles (2-bank "big"
# + 1-bank "small") and each engine reads only its own tile — the Tile dep
# engine serializes concurrent readers of one tile, which would put ACT+DVE
# in series on the PSUM-recycle critical path. Alternating 2,1 ACT-chunks per
# group gives DVE a uniform 50% of keys and balanced engine load.
def _act_split(g):
    return 2 if (g % 2 == 0 or g == 7) else 1


_CACHE = {}


_MAXW = 1  # this walrus build accepts only one sync wait per instruction


def _patch_tile_drain():
    """This walrus build rejects instructions carrying more than one sync
    wait. Tile's sem-assignment freely puts several waits on one instruction
    (and the kernel-tail drain waits on every logical processor). Split the
    excess onto injected same-engine nops placed immediately before the
    overloaded instruction — engines are in-order, so semantics are kept."""
    import concourse.mybir as mybir
    import concourse.tile as tile
    from concourse.vector_clock import ScopedClock

    if getattr(tile.TileContext, "_drain_split_patched", False):
        return

    orig_lower = tile.TileContext._lower_ordered_insts
    counter = [0]

    def _split_waits(self, ordered):
        for bb_name, insts in ordered.items():
            out = []
            for inst in insts:
                si = inst.sync_info
                waits = list(si.on_wait) if si and si.on_wait else []
                if len(waits) > _MAXW:
                    for j in range(0, len(waits) - _MAXW, _MAXW):
                        counter[0] += 1
                        nop = mybir.InstNoOp(name=f"waitsplit-{counter[0]}")
                        nop.engine = inst.engine
                        nop.sync_info = mybir.SyncInfo(
                            on_wait=waits[j:j + _MAXW], on_update=[]
                        )
                        if inst.debug is not None:
                            nop.debug = inst.debug
                        out.append(nop)
                    inst.sync_info = mybir.SyncInfo(
                        on_wait=waits[len(waits) - _MAXW:],
                        on_update=list(si.on_update) if si.on_update else [],
                    )
                out.append(inst)
            ordered[bb_name] = out
        return orig_lower(self, ordered)

    tile.TileContext._lower_ordered_insts = _split_waits

    def _drain_and_barrier(self, tick_clock, wait_clock):
        carrier = self.nc.sync.nop(nofuse=True)
        wait_clock.add_sem_waits(
            carrier.ins, ScopedClock({None: tick_clock.global_clock})
        )
        si = carrier.ins.sync_info
        waits = list(si.on_wait) if si and si.on_wait else []
        carrier.ins.sync_info = mybir.SyncInfo(on_wait=waits[:_MAXW], on_update=[])
        for j in range(_MAXW, len(waits), _MAXW):
            nop = self.nc.sync.nop(nofuse=True)
            nop.ins.sync_info = mybir.SyncInfo(on_wait=waits[j:j + _MAXW], on_update=[])
        self.nc.sync.drain()
        self.nc.all_engine_barrier()
        assert self.sems is not None
        popped = self.nc._tile_sem_poison_stack.pop()
        assert popped is self._sem_poison
        self.nc.clear_and_free_semaphores(list(self.sems.allocated().values()))
        self.nc.all_engine_barrier()

    tile.TileContext._drain_and_barrier = _drain_and_barrier
    tile.TileContext._drain_split_patched = True


def _build():
    import concourse.bass as bass
    import concourse.mybir as mybir
    import concourse.tile as tile
    from concourse.masks import make_identity

    _patch_tile_drain()

    fp32 = mybir.dt.float32
    bf16 = mybir.dt.bfloat16
    i16 = mybir.dt.int16
    Alu = mybir.AluOpType

    nc = bass.Bass("TRN2")
    xT_d = nc.dram_tensor("xT", [D, L1], bf16, kind="ExternalInput")
    sT_d = nc.dram_tensor("srcT", [D, L2], bf16, kind="ExternalInput")
    wq_d = nc.dram_tensor("wq", [128, 512], bf16, kind="ExternalInput")
    wk_d = nc.dram_tensor("wk", [128, 512], bf16, kind="ExternalInput")
    wv_d = nc.dram_tensor("wv", [128, 128], bf16, kind="ExternalInput")
    out_d = nc.dram_tensor("out", [L1, 2 * DH], fp32, kind="ExternalOutput")

    with tile.TileContext(nc) as tc:
        with (
            tc.tile_pool(name="fixed", bufs=1) as fixed,
            tc.tile_pool(name="epool", bufs=20) as epool,
            tc.tile_pool(name="tmp", bufs=3) as tmp,
            tc.tile_pool(name="ps_sc", bufs=2, space="PSUM") as ps_sc,
            tc.tile_pool(name="ps_sm", bufs=2, space="PSUM") as ps_sm,
        ):
            # ---- input DMAs (weights first; big tensors split across queues)
            wq = fixed.tile([128, 512], bf16, tag="wq", name="wq_sb")
            wk = fixed.tile([128, 512], bf16, tag="wk", name="wk_sb")
            wv = fixed.tile([128, 128], bf16, tag="wv", name="wv_sb")
            nc.sync.dma_start(out=wq, in_=wq_d[:, :])
            nc.sync.dma_start(out=wk, in_=wk_d[:, :])
            nc.sync.dma_start(out=wv, in_=wv_d[:, :])
            # big inputs split into quarter tiles so early consumers don't
            # serialize behind later DMA writes (tile-granularity deps); DMAs
            # emitted j-major so each quarter's (t0, t1) pair lands together.
            xT = [[fixed.tile([128, L1 // 4], bf16, tag=f"xT{t}_{j}", name=f"xT{t}_{j}")
                   for j in range(4)] for t in range(2)]
            sT = [[fixed.tile([128, L2 // 4], bf16, tag=f"sT{t}_{j}", name=f"sT{t}_{j}")
                   for j in range(4)] for t in range(2)]
            for j in range(4):
                for t in range(2):
                    a = j * (L2 // 4)
                    nc.sync.dma_start(out=sT[t][j], in_=sT_d[t * 128:(t + 1) * 128, a:a + L2 // 4])
                for t in range(2):
                    a = j * (L1 // 4)
                    nc.sync.dma_start(out=xT[t][j], in_=xT_d[t * 128:(t + 1) * 128, a:a + L1 // 4])

            def sT_ap(t, lo, n):
                j, o = lo // (L2 // 4), lo % (L2 // 4)
                return sT[t][j][:, o:o + n]

            def xT_ap(t, lo, n):
                j, o = lo // (L1 // 4), lo % (L1 // 4)
                return xT[t][j][:, o:o + n]

            # bf16 identity blocks at partitions [0:33] and [64:97] for the
            # sum-transposes of the two attnv strips.
            identw = fixed.tile([128, VW], bf16, tag="ident", name="ident")
            make_identity(nc, identw[0:VW, :])
            nc.sync.dma_start(out=identw[64:64 + VW, :], in_=identw[0:VW, :])

            qT = [fixed.tile([128, L1], bf16, tag=f"qT{h}", name=f"qT{h}") for h in range(2)]
            kT = [[fixed.tile([128, QB], bf16, tag=f"kT{h}_{j}", name=f"kT{h}_{j}")
                   for j in range(L2 // QB)] for h in range(2)]
            # [v_h0 | 1 | v_h1 | 1] per k-chunk: head h stationary = cols
            # [VW*h : VW*h+VW], ones at cols 32 and 65.
            vext = fixed.tile([128, NK, 2 * VW], bf16, tag="v", name="v_sb")
            out_sb = fixed.tile([128, L1 // 128, 2 * DH], fp32, tag="osb", name="osb")

            def k_proj(h):
                for c in range(L2 // QB):
                    psk = ps_sm.tile([128, QB], fp32, tag="small", name="ps_small")
                    for t in range(2):
                        nc.tensor.matmul(
                            psk[:, :],
                            wk[:, t * 256 + h * 128: t * 256 + h * 128 + 128],
                            sT_ap(t, c * QB, QB),
                            start=(t == 0), stop=(t == 1),
                        )
                    # alternate copy engine so the prologue pipelines
                    if c % 2 == 0:
                        nc.vector.tensor_copy(kT[h][c], psk[:, :])
                    else:
                        nc.scalar.copy(kT[h][c], psk[:, :])

            def q_proj(h, c):
                psq = ps_sm.tile([128, QB], fp32, tag="small", name="ps_small")
                for t in range(2):
                    nc.tensor.matmul(
                        psq[:, :],
                        wq[:, t * 256 + h * 128: t * 256 + h * 128 + 128],
                        xT_ap(t, c * QB, QB),
                        start=(t == 0), stop=(t == 1),
                    )
                nc.scalar.copy(qT[h][:, c * QB:(c + 1) * QB], psq[:, :])

            def v_proj():
                nc.vector.memset(vext[:, :, DH::VW], 1.0)  # ones cols 32, 65
                for m0 in range(0, NK, 8):
                    psv = ps_sm.tile([128, 8, 64], fp32, tag="small", name="ps_small")
                    # one accumulation group for the whole bank: start=True
                    # clears the full bank on written partitions, so only the
                    # first matmul may carry it.
                    for jj in range(8):
                        m = m0 + jj
                        for t in range(2):
                            nc.tensor.matmul(
                                psv[:, jj, :],
                                sT_ap(t, m * 128, 128),
                                wv[:, t * 64:(t + 1) * 64],
                                start=(jj == 0 and t == 0),
                                stop=(jj == 7 and t == 1),
                                skip_group_check=True,
                            )
                    for hh in range(2):
                        nc.vector.tensor_copy(
                            vext[:, m0:m0 + 8, VW * hh:VW * hh + DH],
                            psv[:, :, DH * hh:DH * hh + DH],
                        )

            def attnv_mms(h, acc, ets, ms):
                for m in ms:
                    s = m & 1
                    etA, etB, split = ets[m // RT]
                    i = m % RT
                    rhs = (etA[:, i * QB:(i + 1) * QB] if i < split
                           else etB[:, (i - split) * QB:(i - split + 1) * QB])
                    nc.tensor.matmul(
                        acc[64 * s:64 * s + VW, :],
                        vext[:, m, VW * h:VW * h + VW],
                        rhs,
                        start=(m < 2), stop=(m >= NK - 2),
                        tile_position=(0, 64 * s),
                        skip_group_check=True,
                    )

            def attnv_tail(h, c, acc):
                # strip partials -> SBUF bf16 (split across ScalarE/VectorE)
                so = tmp.tile([128, QB], bf16, tag="so", name="so_t")
                nc.scalar.copy(so[0:VW, :], acc[0:VW, :])
                nc.vector.tensor_copy(so[64:64 + VW, :], acc[64:64 + VW, :])
                # sum + transpose both strips: accumulating identity matmuls
                # per 128-query block (strip 1 reads at partition 64). Single
                # accumulation group over the whole pst bank: only the first
                # matmul carries start (start clears the full bank on the
                # written partitions).
                pst = ps_sm.tile([128, 4, VW], fp32, tag="small", name="ps_small")
                for s in range(2):
                    for t in range(4):
                        nc.tensor.matmul(
                            pst[:, t, :],
                            so[64 * s:64 * s + VW, t * 128:(t + 1) * 128],
                            identw[64 * s:64 * s + VW, :],
                            start=(s == 0 and t == 0), stop=(s == 1 and t == 3),
                            skip_group_check=True,
                        )
                rec = tmp.tile([128, 4], fp32, tag="rec", name="rec_t")
                nc.vector.reciprocal(rec[:, :], pst[:, :, DH])
                # normalize on ScalarE: activation Copy takes a per-partition
                # scale AP, freeing the DVE for the exp stream
                for t in range(4):
                    nc.scalar.activation(
                        out_sb[:, c * 4 + t, h * DH:(h + 1) * DH],
                        pst[:, t, 0:DH],
                        mybir.ActivationFunctionType.Copy,
                        scale=rec[:, t:t + 1],
                    )

            def chunk(h, c, prev, nxt):
                """Emit one (h, c) chunk: 16 score groups + split exp, with
                the previous chunk's attnv matmul pairs interleaved between
                groups so the in-order PE fills its exp-wait gaps. The NEXT
                chunk's q projection is hoisted into the middle of this one
                so the chunk boundary has no qT dependency bubble."""
                acc = None
                if prev is not None:
                    ph, pc, pets = prev
                    acc = ps_sm.tile([128, QB], fp32, tag="small", name="ps_small")
                ets = []
                for g in range(NG):
                    if acc is not None:
                        attnv_mms(ph, acc, pets, range(RT * g, RT * g + RT))
                    split = _act_split(g)
                    big = ps_sc.tile([128, 2 * QB], fp32, tag="sc2", name="ps_big")
                    sml = ps_sc.tile([128, QB], fp32, tag="sc1", name="ps_sml")
                    # ACT reads `split` k-chunks, DVE the other RT-split; the
                    # bigger share always goes to the 2-bank tile.
                    if split == 2:
                        dsts = [big[:, 0:QB], big[:, QB:2 * QB], sml[:, :]]
                        act_src, dve_src = big[:, :], sml[:, :]
                    else:
                        dsts = [sml[:, :], big[:, 0:QB], big[:, QB:2 * QB]]
                        act_src, dve_src = sml[:, :], big[:, :]
                    for i in range(RT):
                        m = g * RT + i
                        nc.tensor.matmul(
                            dsts[i],
                            kT[h][m // 4][32 * i:32 * i + 32, (m % 4) * 128:(m % 4 + 1) * 128],
                            qT[h][32 * i:32 * i + 32, c * QB:(c + 1) * QB],
                            start=True, stop=True,
                            tile_position=(32 * i, 0),
                        )
                    etA = epool.tile([128, 2 * QB], bf16, tag="EA", name="eA_t")
                    etB = epool.tile([128, 2 * QB], bf16, tag="EB", name="eB_t")
                    nc.scalar.activation(
                        etA[:, 0:split * QB], act_src,
                        mybir.ActivationFunctionType.Exp, scale=float(SCALE),
                    )
                    nc.vector.tensor_scalar(
                        etB[:, 0:(RT - split) * QB].bitcast(i16), dve_src,
                        float(SCALE * SA), float(SB_C),
                        Alu.mult, Alu.add,
                    )
                    ets.append((etA, etB, split))
                    if g == 8 and nxt is not None:
                        q_proj(*nxt)
                    if h == 0 and c == 0 and g < 12:
                        # k projection for head 1, one column block per slot
                        # (chunk (0,0) has no live attnv acc, so the ps_sm
                        # ring is free)
                        psk = ps_sm.tile([128, QB], fp32, tag="small", name="ps_small")
                        for t in range(2):
                            nc.tensor.matmul(
                                psk[:, :],
                                wk[:, t * 256 + 128:t * 256 + 256],
                                sT_ap(t, g * QB, QB),
                                start=(t == 0), stop=(t == 1),
                            )
                        if g % 2 == 0:
                            nc.vector.tensor_copy(kT[1][g], psk[:, :])
                        else:
                            nc.scalar.copy(kT[1][g], psk[:, :])
                if acc is not None:
                    attnv_tail(ph, pc, acc)
                if h == 0 and c == 0:
                    v_proj()
                return (h, c, ets)

            # ---- software-pipelined main loop: attnv runs one chunk behind,
            # its matmuls interleaved into the next chunk's score groups.
            k_proj(0)
            q_proj(0, 0)
            order = [(h, c) for h in range(2) for c in range(NQ)]
            prev = None
            for i, (h, c) in enumerate(order):
                prev = chunk(h, c, prev, order[i + 1] if i + 1 < len(order) else None)
            ph, pc, pets = prev
            facc = ps_sm.tile([128, QB], fp32, tag="small", name="ps_small")
            attnv_mms(ph, facc, pets, range(NK))
            attnv_tail(ph, pc, facc)

            out_r = out_d.rearrange("(b p) o -> p b o", p=128)
            nc.sync.dma_start(out=out_r, in_=out_sb[:, :, :])
    return nc


def _shard_inputs(x, source, Wq, Wk, Wv):
    """Build the 8 per-core input maps (host-side layout prep only)."""
    import ml_dtypes

    bf = ml_dtypes.bfloat16
    x = np.asarray(x, np.float32)
    source = np.asarray(source, np.float32)
    WqT = np.asarray(Wq, np.float32).T.copy()   # [in, out]
    WkT = np.asarray(Wk, np.float32).T.copy()
    WvT = np.asarray(Wv, np.float32).T.copy()

    def pack_rep4(WT, h1, h2):
        rep = np.concatenate(
            [np.tile(WT[:, h * DH:(h + 1) * DH], (1, 4)) for h in (h1, h2)], axis=1
        )  # [256, 256]
        return np.ascontiguousarray(
            rep.reshape(2, 128, 256).transpose(1, 0, 2).reshape(128, 512)
        ).astype(bf)

    def pack_v(WT, h1, h2):
        pair = np.concatenate(
            [WT[:, h * DH:(h + 1) * DH] for h in (h1, h2)], axis=1
        )  # [256, 64]
        return np.ascontiguousarray(
            pair.reshape(2, 128, 64).transpose(1, 0, 2).reshape(128, 128)
        ).astype(bf)

    in_maps = []
    for c in range(N_CORES):
        b, hp = c // 4, c % 4
        h1, h2 = 2 * hp, 2 * hp + 1
        in_maps.append({
            "xT": np.ascontiguousarray(x[b].T).astype(bf),
            "srcT": np.ascontiguousarray(source[b].T).astype(bf),
            "wq": pack_rep4(WqT, h1, h2),
            "wk": pack_rep4(WkT, h1, h2),
            "wv": pack_v(WvT, h1, h2),
        })
    return in_maps


def _gather(results):
    out = np.empty((B, L1, D), np.float32)
    for c in range(N_CORES):
        b, hp = c // 4, c % 4
        out[b, :, hp * 64:(hp + 1) * 64] = results[c]["out"]
    return out


def kernel(x, source, Wq, Wk, Wv):
    import sys
    if "/opt/trn_rl_repo" not in sys.path:
        sys.path.insert(0, "/opt/trn_rl_repo")
    from concourse import bass_utils

    if "nc" not in _CACHE:
        _CACHE["nc"] = _build()
    in_maps = _shard_inputs(x, source, Wq, Wk, Wv)
    res = bass_utils.run_bass_kernel_spmd(
        _CACHE["nc"], in_maps, core_ids=list(range(N_CORES))
    )
    return _gather(res.results)
